# revision 38
# baseline (speedup 1.0000x reference)
"""Self-contained Trainium2 Bass kernel: 16-head attention with RoPE (B=2, S=2048, D=2048).

Sharding: 8 cores = 2 (batch) x 4 (head groups of 4 heads / 512 cols).
Per core: QKV projections for its head group -> RoPE -> causal attention ->
AllGather of attention outputs (X^T) within the 4-core batch group ->
column-sharded output projection. Host assembles by concatenation only.

Pipeline over 4 q-chunks of 512:
  chunk c: V(s-tiles 4c..4c+3), per head {Q(c)+RoPE, K(c)+RoPE} interleaved,
           attention(c) [causal: k-tiles 0..4c+3], AllGather(c),
           outproj scheduled late so collectives overlap compute.

Dataflow is fully "transposed" so no on-chip transposes are needed:
  hiddenT [d, s] (host-pretransposed, bf16), streamed per chunk
  QT/KT   [dh, s] per head  (projection emits head-dim-major directly;
          K is stored in per-chunk tiles to avoid false RoPE deps)
  S^T     [k, q] scores, two k-tiles paired in one [128,1024] PSUM tile
  P^T     [k, q] = exp(S^T + mask^T), one ACT op per pair
  row-sums accumulated in SBUF split across Vector/GpSimd engines,
  partition-reduced+broadcast with one all-ones matmul
  O^T     [dh, q] = V^T @ P^T          (lhsT = V natural [s, dh])
  X^T     AllGather on first axis (last chunk split by head pairs)
  out     [s, oc] (lhsT = X^T block, rhs = WoT)
RoPE de-interleave is folded into a host-side row permutation of Wq/Wk;
RoPE itself is 3 full-height vector ops with sign-folded sin tables.
1/sqrt(DH) is folded into the Q rope tables.
"""

import math
from contextlib import ExitStack

import numpy as np
import ml_dtypes

B, S, D, H, DH = 2, 2048, 2048, 16, 128
NCORES = 8
GPC = 4            # cores per tensor-parallel group
HPC = H // GPC     # heads per core (4)
CW = HPC * DH      # 512 columns per core
NEG = -1e9
BF = ml_dtypes.bfloat16
QCH = 512          # q-chunk (moving free dim)
NQC = S // QCH     # 4
NDT = D // 128     # 16 d-tiles
NST = S // 128     # 16 s-tiles

REPLICA_GROUPS = [[0, 1, 2, 3], [4, 5, 6, 7]]

_built = {}


def _build(mode: str, use_bias: bool):
    # mode: 'causal' (standard causal mask), 'zero' (mask is all zeros),
    #       'general' (arbitrary additive mask, streamed from DRAM)
    import concourse.bass as bass
    import concourse.tile as tile
    from concourse import bacc, bass_isa, mybir
    from concourse.tile_rust import add_dep_helper

    causal = mode == 'causal'

    f32, bf16 = mybir.dt.float32, mybir.dt.bfloat16
    EXP = mybir.ActivationFunctionType.Exp
    IDN = mybir.ActivationFunctionType.Identity

    nc = bacc.Bacc("TRN2", target_bir_lowering=False, debug=False,
                   num_devices=NCORES)

    # weights and hidden states arrive host-packed as [128, tiles*cols] walls
    # so each loads with a handful of large DMAs (the SP sequencer issues
    # DMA triggers at ~650ns each -- many small DMAs throttle startup)
    WW = NDT * CW      # 8192 wall columns per weight matrix
    HW = NDT * QCH     # 8192 wall columns per hidden-state chunk
    hT_d = nc.dram_tensor("hiddenT", [128, NQC * HW], bf16, kind="ExternalInput")
    wq_d = nc.dram_tensor("wqT", [128, WW], bf16, kind="ExternalInput")
    wk_d = nc.dram_tensor("wkT", [128, WW], bf16, kind="ExternalInput")
    wv_d = nc.dram_tensor("wvT", [128, WW], bf16, kind="ExternalInput")
    wo_d = nc.dram_tensor("woT", [128, WW], bf16, kind="ExternalInput")
    cq_d = nc.dram_tensor("cq", [128, S], bf16, kind="ExternalInput")
    sq_d = nc.dram_tensor("sq2", [128, S], bf16, kind="ExternalInput")
    ck_d = nc.dram_tensor("ck", [128, S], bf16, kind="ExternalInput")
    sk_d = nc.dram_tensor("sk2", [128, S], bf16, kind="ExternalInput")
    pw_d = nc.dram_tensor("pswap", [128, 128], bf16, kind="ExternalInput")
    if use_bias:
        bq_d = nc.dram_tensor("bqp", [128, HPC], f32, kind="ExternalInput")
        bk_d = nc.dram_tensor("bkp", [128, HPC], f32, kind="ExternalInput")
        bv_d = nc.dram_tensor("bv2", [1, CW], f32, kind="ExternalInput")
        bo_d = nc.dram_tensor("bo2", [1, CW], f32, kind="ExternalInput")
    if causal:
        dm_d = nc.dram_tensor("dmask", [128, 128], bf16, kind="ExternalInput")
    elif mode == 'general':
        mT_d = nc.dram_tensor("maskT", [S, S], bf16, kind="ExternalInput")
    out_d = nc.dram_tensor("out", [S, CW], bf16, kind="ExternalOutput")

    with tile.TileContext(nc) as tc, ExitStack() as ctx:
        hp = ctx.enter_context(tc.tile_pool(name="hp", bufs=2 * NDT + 4))
        xp = ctx.enter_context(tc.tile_pool(name="xp", bufs=NDT))
        wp = ctx.enter_context(tc.tile_pool(name="wp", bufs=4 * NDT))
        qkp = ctx.enter_context(tc.tile_pool(name="qkp", bufs=2 * HPC))
        vp = ctx.enter_context(tc.tile_pool(name="vp", bufs=NST))
        cst = ctx.enter_context(tc.tile_pool(name="cst", bufs=1))
        ptp = ctx.enter_context(tc.tile_pool(name="ptp", bufs=4))
        rp = ctx.enter_context(tc.tile_pool(name="rp", bufs=2))
        op = ctx.enter_context(tc.tile_pool(name="op", bufs=3))
        # PSUM: ss pairs 2x[128,1024] = 4 banks, pv 2x[128,512] = 2 banks,
        # mm (proj+outproj shared) 2x[128,512] = 2 banks
        ps_ss = ctx.enter_context(tc.tile_pool(name="ps_ss", bufs=2, space="PSUM"))
        ps_pv = ctx.enter_context(tc.tile_pool(name="ps_pv", bufs=2, space="PSUM"))
        ps_mm = ctx.enter_context(tc.tile_pool(name="ps_mm", bufs=2, space="PSUM"))
        dram = ctx.enter_context(tc.tile_pool(name="dram", bufs=1, space="DRAM"))

        # ---- startup barrier: absorb host-side core-launch skew here (it
        # overlaps the DMA-bound weight load) so the first real AllGather
        # doesn't swallow it mid-pipeline, freezing kernel DMA for ~40us.
        bar_in = dram.tile([1, 128], bf16, tag="bar_in", name="bar_in")
        bar_out = dram.tile([GPC, 128], bf16, tag="bar_out", name="bar_out")
        bar_sb = cst.tile([1, 128], bf16, tag="bar", name="bar_sb")
        nc.vector.memset(bar_sb[:], 0.0)
        nc.sync.dma_start(bar_in[:], bar_sb[:])
        nc.gpsimd.collective_compute(
            "AllGather", mybir.AluOpType.bypass,
            replica_groups=REPLICA_GROUPS,
            ins=[bar_in[:].opt()], outs=[bar_out[:].opt()])

        # ---- first-needed data first: Wv + hT(chunk 0), then Wq, Wk ----
        NW = 8              # DMAs per wall (256KB each: good issue/bw balance)
        WSTEP, HSTEP = WW // NW, HW // NW
        wv_sb = wp.tile([128, WW], bf16, tag="wv", name="wv_sb", bufs=1)
        wq_sb = wp.tile([128, WW], bf16, tag="wq", name="wq_sb", bufs=1)
        wk_sb = wp.tile([128, WW], bf16, tag="wk", name="wk_sb", bufs=1)
        wo_sb = wp.tile([128, WW], bf16, tag="wo", name="wo_sb", bufs=1)
        hTc0 = hp.tile([128, HW], bf16, tag="hT", name="hT0", bufs=2)
        for i in range(NW):
            if i == 0:
                # first pieces split in half so the first matmul can start
                # ~6us earlier
                h = WSTEP // 2
                nc.sync.dma_start(wv_sb[:, 0:h], wv_d[:, 0:h])
                nc.sync.dma_start(hTc0[:, 0:h], hT_d[:, 0:h])
                nc.sync.dma_start(wv_sb[:, h:WSTEP], wv_d[:, h:WSTEP])
                nc.sync.dma_start(hTc0[:, h:HSTEP], hT_d[:, h:HSTEP])
                continue
            nc.sync.dma_start(wv_sb[:, i * WSTEP:(i + 1) * WSTEP],
                              wv_d[:, i * WSTEP:(i + 1) * WSTEP])
            nc.sync.dma_start(hTc0[:, i * HSTEP:(i + 1) * HSTEP],
                              hT_d[:, i * HSTEP:(i + 1) * HSTEP])

        # ---- constants + Q/K weights (Q first: Q-proj runs before K-proj) ----
        cq_sb = cst.tile([128, S], bf16, tag="cq", name="cq_sb")
        sq_sb = cst.tile([128, S], bf16, tag="sq", name="sq_sb")
        ck_sb = cst.tile([128, S], bf16, tag="ck", name="ck_sb")
        sk_sb = cst.tile([128, S], bf16, tag="sk", name="sk_sb")
        for i in range(NW):
            nc.sync.dma_start(wq_sb[:, i * WSTEP:(i + 1) * WSTEP],
                              wq_d[:, i * WSTEP:(i + 1) * WSTEP])
        nc.sync.dma_start(cq_sb[:], cq_d[:])
        nc.sync.dma_start(sq_sb[:], sq_d[:])
        for i in range(NW):
            nc.sync.dma_start(wk_sb[:, i * WSTEP:(i + 1) * WSTEP],
                              wk_d[:, i * WSTEP:(i + 1) * WSTEP])
        nc.sync.dma_start(ck_sb[:], ck_d[:])
        nc.sync.dma_start(sk_sb[:], sk_d[:])
        if use_bias:
            bq_sb = cst.tile([128, HPC], f32, tag="bq", name="bq_sb")
            bk_sb = cst.tile([128, HPC], f32, tag="bk", name="bk_sb")
            bv_sb = cst.tile([1, CW], f32, tag="bv", name="bv_sb")
            bo_sb = cst.tile([1, CW], f32, tag="bo", name="bo_sb")
            nc.sync.dma_start(bq_sb[:], bq_d[:])
            nc.sync.dma_start(bk_sb[:], bk_d[:])
            nc.sync.dma_start(bv_sb[:], bv_d[:])
            nc.sync.dma_start(bo_sb[:], bo_d[:])
            bvb_sb = cst.tile([128, CW], f32, tag="bvb", name="bvb_sb")
            bob_sb = cst.tile([128, CW], f32, tag="bob", name="bob_sb")
            nc.gpsimd.partition_broadcast(bvb_sb[:], bv_sb[0:1, :])
            nc.gpsimd.partition_broadcast(bob_sb[:], bo_sb[0:1, :])
        ones_sb = cst.tile([128, 128], bf16, tag="ones", name="ones_sb")
        nc.vector.memset(ones_sb[:], 1.0)
        psw_sb = cst.tile([128, 128], bf16, tag="psw", name="psw_sb")
        nc.sync.dma_start(psw_sb[:], pw_d[:])
        if causal:
            tri_sb = cst.tile([128, 128], bf16, tag="tri", name="tri_sb")
            nc.sync.dma_start(tri_sb[:], dm_d[:])

        # Wo streams in behind everything else
        for i in range(NW):
            nc.sync.dma_start(wo_sb[:, i * WSTEP:(i + 1) * WSTEP],
                              wo_d[:, i * WSTEP:(i + 1) * WSTEP])

        # per-(chunk, head) K tiles: fresh tile per chunk so old-chunk reads
        # never depend on the current chunk's RoPE
        ktc = [[None] * HPC for _ in range(NQC)]
        v_sb = [None] * NST

        def rope(dst, ps, c_sb, s_sb, c, b_ap, prefix):
            """dst = rope(ps): ACT copy, then a matmul-swap (rotate rows by
            64 via a permutation stationary -- no DMA, so a collective's
            DMA-freeze can never stall the rope chain) + 3 vector ops.
            rows 0:64 = a (real), 64:128 = b (imag);
            c_sb rows = [cos;cos], s_sb rows = [-sin;+sin]."""
            csl = slice(c * QCH, (c + 1) * QCH)
            if use_bias:
                nc.scalar.activation(dst[:], ps[:], IDN, bias=b_ap)
            else:
                nc.scalar.activation(dst[:], ps[:], IDN)
            t1 = ps_pv.tile([128, QCH], f32, tag="pv", name=f"{prefix}t1")
            nc.tensor.matmul(t1[:], psw_sb[:], dst[:], start=True, stop=True)
            t2 = rp.tile([128, QCH], bf16, tag="t2", name=f"{prefix}t2")
            nc.vector.tensor_mul(t2[:], dst[:], c_sb[:, csl])
            nc.vector.tensor_mul(dst[:], t1[:], s_sb[:, csl])
            nc.vector.tensor_add(dst[:], dst[:], t2[:])

        def proj_head(w_sb, b_sb, c_sb, s_sb, dst, hTc, c, m, prefix):
            ps = ps_mm.tile([128, QCH], f32, tag="mm", name=f"{prefix}ps{m}_{c}")
            mm = None
            for dt in range(NDT):
                mm = nc.tensor.matmul(
                    ps[:], w_sb[:, dt * CW + m * 128:dt * CW + (m + 1) * 128],
                    hTc[:, dt * QCH:(dt + 1) * QCH],
                    start=(dt == 0), stop=(dt == NDT - 1))
            return mm, (dst, ps, c_sb, s_sb, c,
                        b_sb[:, m:m + 1] if use_bias else None,
                        f"{prefix}{m}_{c}")

        hTcs = {0: hTc0}

        def proj_block(c):
            if c + 1 < NQC:
                nxt = hp.tile([128, HW], bf16, tag="hT", name=f"hT{c + 1}",
                              bufs=2)
                for i in range(4):   # mid-kernel: fewer, bigger DMAs
                    st = HW // 4
                    nc.sync.dma_start(
                        nxt[:, i * st:(i + 1) * st],
                        hT_d[:, (c + 1) * HW + i * st:(c + 1) * HW +
                             (i + 1) * st])
                hTcs[c + 1] = nxt
            hTc = hTcs[c]
            # V for s-tiles 4c..4c+3
            for sti in range(4):
                st = 4 * c + sti
                ps = ps_mm.tile([128, CW], f32, tag="mm", name=f"psv{st}")
                for dt in range(NDT):
                    nc.tensor.matmul(
                        ps[:],
                        hTc[:, dt * QCH + sti * 128:dt * QCH + (sti + 1) * 128],
                        wv_sb[:, dt * CW:(dt + 1) * CW],
                        start=(dt == 0), stop=(dt == NDT - 1))
                vt = vp.tile([128, CW], bf16, tag="v", name=f"v{st}")
                if use_bias:
                    nc.vector.tensor_add(vt[:], ps[:], bvb_sb[:])
                else:
                    nc.scalar.activation(vt[:], ps[:], IDN)
                v_sb[st] = vt
            # Q and K interleaved per head so the ACT copies never pile up
            # in front of attention's exps, and head 0 is ready first.
            # Rope emission is deferred one projection so its swap-matmul
            # (which waits on the ACT copy) never stalls the PE FIFO.
            qtrc = []
            plast = None
            pend_rope = None
            for m in range(HPC):
                qt = qkp.tile([128, QCH], bf16, tag="qtc", name=f"qtc{c}_{m}",
                              bufs=HPC + 1 if causal else NQC * HPC)
                _, rq = proj_head(wq_sb, bq_sb if use_bias else None,
                                  cq_sb, sq_sb, qt, hTc, c, m, "q")
                if pend_rope is not None:
                    rope(*pend_rope)
                pend_rope = rq
                qtrc.append(qt)
                kt = qkp.tile([128, QCH], bf16, tag="kt", name=f"kt{c}_{m}",
                              bufs=NQC * HPC)
                plast, rk = proj_head(wk_sb, bk_sb if use_bias else None,
                                      ck_sb, sk_sb, kt, hTc, c, m, "k")
                rope(*pend_rope)
                pend_rope = rk
                ktc[c][m] = kt
            rope(*pend_rope)
            return qtrc, plast

        def attention_chunk(c, qtrc):
            nk = 4 * c + 4 if causal else NST
            npair = nk // 2
            # AllGather pieces: last causal chunk split by head pairs so the
            # first collective fires mid-attention.
            split = (c == NQC - 1)
            if split:
                agins = [dram.tile([2 * 128, QCH], bf16, tag=f"agin{c}_{i}",
                                   name=f"agin{c}_{i}") for i in range(2)]
                agouts_l = [dram.tile([GPC * 2 * 128, QCH], bf16,
                                      tag=f"agout{c}_{i}",
                                      name=f"agout{c}_{i}") for i in range(2)]
            else:
                agins = [dram.tile([CW, QCH], bf16, tag=f"agin{c}_0",
                                   name=f"agin{c}_0")]
                agouts_l = [dram.tile([D, QCH], bf16, tag=f"agout{c}_0",
                                      name=f"agout{c}_0")]

            state = {}  # h -> (pv, se, so)
            pend = None  # (h, pr, pt, info)
            last_mm = [None]

            def emit_scores(h, pr):
                ss = ps_ss.tile([128, 2 * QCH], f32, tag="ss",
                                name=f"ss{c}_{h}_{pr}")
                info = []
                for j in range(2):
                    ki = 2 * pr + j
                    p = ki - 4 * c if causal else -1
                    c0 = max(0, 128 * p)
                    cc, col = divmod(ki, 4)
                    nc.tensor.matmul(
                        ss[:, j * QCH + c0:(j + 1) * QCH],
                        ktc[cc][h][:, col * 128:(col + 1) * 128],
                        qtrc[h][:, c0:], start=True, stop=True)
                    if causal and p >= 0:
                        nc.vector.tensor_add(
                            ss[:, j * QCH + c0:j * QCH + c0 + 128],
                            ss[:, j * QCH + c0:j * QCH + c0 + 128], tri_sb[:])
                    info.append((ki, c0))
                if mode == 'general':
                    mt = ptp.tile([128, 2 * QCH], bf16, tag="mt",
                                  name=f"mt{c}_{h}_{pr}", bufs=3)
                    for j in range(2):
                        ki = 2 * pr + j
                        nc.sync.dma_start(
                            mt[:, j * QCH:(j + 1) * QCH],
                            mT_d[ki * 128:(ki + 1) * 128,
                                 c * QCH:(c + 1) * QCH])
                    nc.vector.tensor_add(ss[:], ss[:], mt[:])
                pt = ptp.tile([128, 2 * QCH], bf16, tag="pt",
                              name=f"pt{c}_{h}_{pr}", bufs=4)
                e0 = info[0][1]   # skip fully-masked leading columns
                nc.scalar.activation(pt[:, e0:], ss[:, e0:], EXP)
                return pt, info

            def emit_pv(h, pr, pt, info):
                if h not in state:
                    pv = ps_pv.tile([128, QCH], f32, tag="pv", name=f"pv{c}_{h}")
                    sa = rp.tile([128, QCH], bf16, tag="sa", name=f"sa{c}_{h}",
                                 bufs=2)
                    state[h] = (pv, sa)
                pv, sa = state[h]
                for j, (ki, c0) in enumerate(info):
                    src = pt[:, j * QCH + c0:(j + 1) * QCH]
                    if ki == 0:
                        nc.vector.tensor_copy(sa[:], src)
                    else:
                        nc.vector.tensor_add(sa[:, c0:], sa[:, c0:], src)
                    last_mm[0] = nc.tensor.matmul(
                        pv[:, c0:], v_sb[ki][:, h * 128:(h + 1) * 128],
                        src, start=(ki == 0), stop=(ki == nk - 1),
                        skip_group_check=(c0 > 0))

            def emit_tail(h):
                pv, sa = state[h]
                # partition-reduce+broadcast the row sums with one all-ones
                # matmul (GpSimd partition_all_reduce measures 3.5us -- too
                # slow); the result tile squats in an ss-pool slot.
                smt = ps_ss.tile([128, 2 * QCH], f32, tag="ss",
                                 name=f"sm{c}_{h}")
                nc.tensor.matmul(smt[:, 0:QCH], ones_sb[:], sa[:],
                                 start=True, stop=True)
                recb = op.tile([128, QCH], f32, tag="recb",
                               name=f"recb{c}_{h}", bufs=2)
                nc.vector.reciprocal_approx_fast(out=recb[:], in_=smt[:, 0:QCH])
                ot = op.tile([128, QCH], bf16, tag="ot", name=f"ot{c}_{h}",
                             bufs=3)
                nc.vector.tensor_mul(ot[:], pv[:], recb[:])
                if split:
                    nc.sync.dma_start(agins[h // 2][(h % 2) * 128:
                                                    (h % 2 + 1) * 128, :], ot[:])
                    if h % 2 == 1:
                        nc.gpsimd.collective_compute(
                            "AllGather", mybir.AluOpType.bypass,
                            replica_groups=REPLICA_GROUPS,
                            ins=[agins[h // 2][:].opt()],
                            outs=[agouts_l[h // 2][:].opt()])
                else:
                    nc.sync.dma_start(agins[0][h * 128:(h + 1) * 128, :], ot[:])

            units = [(h, pr) for h in range(HPC) for pr in range(npair)]
            for h, pr in units:
                cur = (h, pr, *emit_scores(h, pr))
                if pend is not None:
                    emit_pv(pend[0], pend[1], pend[2], pend[3])
                    if pend[1] == npair - 1:
                        emit_tail(pend[0])
                pend = cur
            emit_pv(pend[0], pend[1], pend[2], pend[3])
            emit_tail(pend[0])
            if not split:
                nc.gpsimd.collective_compute(
                    "AllGather", mybir.AluOpType.bypass,
                    replica_groups=REPLICA_GROUPS,
                    ins=[agins[0][:].opt()], outs=[agouts_l[0][:].opt()])
            # pieces: list of (agout, [global d-tile per 128-row block])
            if split:
                pieces = [(agouts_l[i],
                           [4 * (j // 2) + 2 * i + (j % 2) for j in range(8)])
                          for i in range(2)]
            else:
                pieces = [(agouts_l[0], list(range(NDT)))]
            return pieces, last_mm[0]

        def outproj_chunk(qc, pieces, after_mm=None):
            xt = []   # list per piece
            for pi, (agout, dts) in enumerate(pieces):
                tiles = []
                for j, dt in enumerate(dts):
                    t = xp.tile([128, QCH], bf16, tag="xt",
                                name=f"xt{qc}_{pi}_{j}")
                    nc.sync.dma_start(t[:], agout[j * 128:(j + 1) * 128, :])
                    tiles.append((dt, t))
                xt.append(tiles)
            npieces = len(xt)
            nst4 = QCH // 128
            # For the split (last) chunk: run all piece-0-dependent matmuls
            # across all four output groups before any piece-1 matmul, so
            # the first AllGather's work fills the second one's latency.
            # The four concurrent PSUM groups squat in the idle ss pool.
            if npieces > 1:
                pss = [ps_ss.tile([128, 2 * QCH], f32, tag="ss",
                                  name=f"pso{qc}_{g}") for g in range(2)]
                groups = [pss[g][:, half * QCH:(half + 1) * QCH]
                          for g in (0, 1) for half in (0, 1)]
            else:
                groups = [None] * nst4
            for pi in range(npieces):
                last_piece = pi == npieces - 1
                for st4 in range(nst4):
                    if groups[st4] is None:
                        groups[st4] = ps_mm.tile([128, CW], f32, tag="mm",
                                                 name=f"pso{qc}_{st4}")[:]
                    n = len(xt[pi])
                    for i, (dt, t) in enumerate(xt[pi]):
                        mm = nc.tensor.matmul(
                            groups[st4][:], t[:, st4 * 128:(st4 + 1) * 128],
                            wo_sb[:, dt * CW:(dt + 1) * CW],
                            start=(pi == 0 and i == 0),
                            stop=(last_piece and i == n - 1))
                        if after_mm is not None:
                            # keep outproj(qc) late in PE program order: the
                            # static scheduler underestimates AllGather
                            # latency and would hoist it otherwise.
                            add_dep_helper(mm.ins, after_mm.ins, sync=False,
                                           reason="outproj ordering")
                            after_mm = None
                    if not last_piece:
                        continue
                    row = qc * QCH + st4 * 128
                    of = op.tile([128, CW], bf16, tag="of",
                                 name=f"of{qc}_{st4}", bufs=2)
                    if use_bias:
                        nc.vector.tensor_add(of[:], groups[st4][:], bob_sb[:])
                    else:
                        nc.scalar.activation(of[:], groups[st4][:], IDN)
                    nc.sync.dma_start(out_d[row:row + 128, :], of[:])

        # ---- main pipeline ----
        # outproj(c) runs two phases after its AllGather fires so even a
        # slow collective never blocks the PE queue:
        # proj0 attn0 / proj1 attn1 / proj2 op0 attn2 / proj3 op1 attn3 /
        # op2 op3
        pieces, last_mms = {}, {}
        if causal:
            for c in range(NQC):
                qtrc, plast = proj_block(c)
                if c >= 2:
                    outproj_chunk(c - 2, pieces[c - 2], plast)
                pieces[c], last_mms[c] = attention_chunk(c, qtrc)
            outproj_chunk(NQC - 2, pieces[NQC - 2], last_mms[NQC - 1])
            outproj_chunk(NQC - 1, pieces[NQC - 1], last_mms[NQC - 1])
        else:
            # non-causal: attention(c) needs the FULL K/V, so project
            # everything first, then run the attention/AG/outproj pipeline
            qtrcs = [proj_block(c)[0] for c in range(NQC)]
            for c in range(NQC):
                pieces[c], last_mms[c] = attention_chunk(c, qtrcs[c])
                if c > 0:
                    outproj_chunk(c - 1, pieces[c - 1], last_mms[c])
            outproj_chunk(NQC - 1, pieces[NQC - 1], last_mms[NQC - 1])

    nc.compile()
    return nc


def _get_built(mode: str, use_bias: bool):
    key = (mode, use_bias)
    if key not in _built:
        _built[key] = _build(mode, use_bias)
    return _built[key]


def _prep_inputs(inputs, mode, use_bias):
    hs = np.asarray(inputs["hidden_states"], np.float32)
    fc = np.asarray(inputs["freqs_cis"], np.float32)
    Wq = np.asarray(inputs["Wq"], np.float32)
    Wk = np.asarray(inputs["Wk"], np.float32)
    Wv = np.asarray(inputs["Wv"], np.float32)
    Wo = np.asarray(inputs["Wo"], np.float32)
    bq = np.asarray(inputs["bq"], np.float32)
    bk = np.asarray(inputs["bk"], np.float32)
    bv = np.asarray(inputs["bv"], np.float32)
    bo = np.asarray(inputs["bo"], np.float32)

    # de-interleave permutation per 128-row head block: [0,2,..,126, 1,3,..,127]
    perm1 = np.concatenate([np.arange(0, DH, 2), np.arange(1, DH, 2)])
    permC = (np.arange(CW) // DH) * DH  # head base offsets
    perm = permC + perm1[np.arange(CW) % DH]

    scale = 1.0 / math.sqrt(DH)
    cos = np.concatenate([fc[:, :, 0].T, fc[:, :, 0].T])  # [128, S], dup halves
    sinT = fc[:, :, 1].T
    sin2 = np.concatenate([-sinT, sinT])                  # sign-folded
    cq = np.ascontiguousarray(cos * scale).astype(BF)
    sq2 = np.ascontiguousarray(sin2 * scale).astype(BF)
    ck = np.ascontiguousarray(cos).astype(BF)
    sk2 = np.ascontiguousarray(sin2).astype(BF)

    pswap = np.zeros((128, 128), np.float32)
    pswap[(np.arange(128) + 64) % 128, np.arange(128)] = 1.0
    pswap = pswap.astype(BF)

    if mode == 'causal':
        tri = np.where(np.arange(128)[:, None] > np.arange(128)[None, :],
                       np.float32(NEG), np.float32(0.0)).astype(BF)
    elif mode == 'general':
        maskT = np.ascontiguousarray(
            np.asarray(inputs["mask"], np.float32)[0, 0].T).astype(BF)

    def pack_w(wT):
        # [D, CW] -> [128, NDT*CW] wall (d-tile rows side by side)
        return np.ascontiguousarray(
            np.concatenate([wT[dt * 128:(dt + 1) * 128, :]
                            for dt in range(NDT)], axis=1)).astype(BF)

    def pack_h(hsT):
        # [D, S] -> [128, NQC*NDT*QCH] wall, chunk-major then d-tile
        segs = [hsT[dt * 128:(dt + 1) * 128, c * QCH:(c + 1) * QCH]
                for c in range(NQC) for dt in range(NDT)]
        return np.ascontiguousarray(np.concatenate(segs, axis=1)).astype(BF)

    hTb = [pack_h(hs[b].T) for b in range(B)]

    in_maps = []
    for c in range(NCORES):
        b, hg = divmod(c, GPC)
        sl = slice(CW * hg, CW * (hg + 1))
        wq_s = Wq[sl][perm]
        wk_s = Wk[sl][perm]
        m = {
            "hiddenT": hTb[b],
            "wqT": pack_w(wq_s.T),
            "wkT": pack_w(wk_s.T),
            "wvT": pack_w(Wv[sl].T),
            "woT": pack_w(Wo[sl].T),
            "cq": cq, "sq2": sq2, "ck": ck, "sk2": sk2, "pswap": pswap,
        }
        if use_bias:
            m["bqp"] = np.ascontiguousarray(
                bq[sl][perm].reshape(HPC, 128).T).astype(np.float32)
            m["bkp"] = np.ascontiguousarray(
                bk[sl][perm].reshape(HPC, 128).T).astype(np.float32)
            m["bv2"] = bv[sl].reshape(1, CW).astype(np.float32)
            m["bo2"] = bo[sl].reshape(1, CW).astype(np.float32)
        if mode == 'causal':
            m["dmask"] = tri
        elif mode == 'general':
            m["maskT"] = maskT
        in_maps.append(m)
    return in_maps


def _mask_mode(mask):
    mask = np.asarray(mask, np.float32)
    if mask.shape != (1, 1, S, S):
        return 'general'
    m = mask[0, 0]
    if not np.any(m):
        return 'zero'
    expect = np.triu(np.full((S, S), np.float32(NEG)), k=1)
    if np.array_equal(m, expect):
        return 'causal'
    return 'general'


def run_on_cores(inputs, trace=False):
    """Compile+run; returns BassKernelResults."""
    from concourse.bass_utils import run_bass_kernel_spmd
    mode = _mask_mode(inputs["mask"])
    use_bias = any(
        np.any(np.asarray(inputs[k])) for k in ("bq", "bk", "bv", "bo"))
    nc = _get_built(mode, use_bias)
    in_maps = _prep_inputs(inputs, mode, use_bias)
    r = run_bass_kernel_spmd(nc, in_maps, list(range(NCORES)), trace=trace)
    return r


def kernel(**inputs) -> np.ndarray:
    r = run_on_cores(inputs)
    out = np.empty((B, S, D), np.float32)
    for c in range(NCORES):
        b, hg = divmod(c, GPC)
        out[b, :, CW * hg:CW * (hg + 1)] = \
            np.asarray(r.results[c]["out"]).astype(np.float32)
    return out


# revision 43
# speedup vs baseline: 1.0339x; 1.0339x over previous
"""Self-contained Trainium2 Bass kernel: 16-head attention with RoPE (B=2, S=2048, D=2048).

Sharding: 8 cores = 2 (batch) x 4 (head groups of 4 heads / 512 cols).
Per core: QKV projections for its head group -> RoPE -> causal attention ->
AllGather of attention outputs (X^T) within the 4-core batch group ->
column-sharded output projection. Host assembles by concatenation only.

Pipeline over 4 q-chunks of 512:
  chunk c: V(s-tiles 4c..4c+3), per head {Q(c)+RoPE, K(c)+RoPE} interleaved,
           attention(c) [causal: k-tiles 0..4c+3], AllGather(c),
           outproj scheduled late so collectives overlap compute.

Dataflow is fully "transposed" so no on-chip transposes are needed:
  hiddenT [d, s] (host-pretransposed, bf16), streamed per chunk
  QT/KT   [dh, s] per head  (projection emits head-dim-major directly;
          K is stored in per-chunk tiles to avoid false RoPE deps)
  S^T     [k, q] scores, two k-tiles paired in one [128,1024] PSUM tile
  P^T     [k, q] = exp(S^T + mask^T), one ACT op per pair
  row-sums accumulated in SBUF split across Vector/GpSimd engines,
  partition-reduced+broadcast with one all-ones matmul
  O^T     [dh, q] = V^T @ P^T          (lhsT = V natural [s, dh])
  X^T     AllGather on first axis (last chunk split by head pairs)
  out     [s, oc] (lhsT = X^T block, rhs = WoT)
RoPE de-interleave is folded into a host-side row permutation of Wq/Wk;
RoPE itself is 3 full-height vector ops with sign-folded sin tables.
1/sqrt(DH) is folded into the Q rope tables.
"""

import math
from contextlib import ExitStack

import numpy as np
import ml_dtypes

B, S, D, H, DH = 2, 2048, 2048, 16, 128
NCORES = 8
GPC = 4            # cores per tensor-parallel group
HPC = H // GPC     # heads per core (4)
CW = HPC * DH      # 512 columns per core
NEG = -1e9
BF = ml_dtypes.bfloat16
QCH = 512          # q-chunk (moving free dim)
NQC = S // QCH     # 4
NDT = D // 128     # 16 d-tiles
NST = S // 128     # 16 s-tiles

REPLICA_GROUPS = [[0, 1, 2, 3], [4, 5, 6, 7]]

_built = {}


def _build(mode: str, use_bias: bool):
    # mode: 'causal' (standard causal mask), 'zero' (mask is all zeros),
    #       'general' (arbitrary additive mask, streamed from DRAM)
    import concourse.bass as bass
    import concourse.tile as tile
    from concourse import bacc, bass_isa, mybir
    from concourse.tile_rust import add_dep_helper

    causal = mode == 'causal'

    f32, bf16 = mybir.dt.float32, mybir.dt.bfloat16
    EXP = mybir.ActivationFunctionType.Exp
    IDN = mybir.ActivationFunctionType.Identity

    nc = bacc.Bacc("TRN2", target_bir_lowering=False, debug=False,
                   num_devices=NCORES)

    # weights and hidden states arrive host-packed as [128, tiles*cols] walls
    # so each loads with a handful of large DMAs (the SP sequencer issues
    # DMA triggers at ~650ns each -- many small DMAs throttle startup)
    WW = NDT * CW      # 8192 wall columns per weight matrix
    HW = NDT * QCH     # 8192 wall columns per hidden-state chunk
    hT_d = nc.dram_tensor("hiddenT", [128, NQC * HW], bf16, kind="ExternalInput")
    wq_d = nc.dram_tensor("wqT", [128, WW], bf16, kind="ExternalInput")
    wk_d = nc.dram_tensor("wkT", [128, WW], bf16, kind="ExternalInput")
    wv_d = nc.dram_tensor("wvT", [128, WW], bf16, kind="ExternalInput")
    wo_d = nc.dram_tensor("woT", [128, WW], bf16, kind="ExternalInput")
    cq_d = nc.dram_tensor("cq", [128, S], bf16, kind="ExternalInput")
    sq_d = nc.dram_tensor("sq2", [128, S], bf16, kind="ExternalInput")
    ck_d = nc.dram_tensor("ck", [128, S], bf16, kind="ExternalInput")
    sk_d = nc.dram_tensor("sk2", [128, S], bf16, kind="ExternalInput")
    pw_d = nc.dram_tensor("pswap", [128, 128], bf16, kind="ExternalInput")
    if use_bias:
        bq_d = nc.dram_tensor("bqp", [128, HPC], f32, kind="ExternalInput")
        bk_d = nc.dram_tensor("bkp", [128, HPC], f32, kind="ExternalInput")
        bv_d = nc.dram_tensor("bv2", [1, CW], f32, kind="ExternalInput")
        bo_d = nc.dram_tensor("bo2", [1, CW], f32, kind="ExternalInput")
    if causal:
        dm_d = nc.dram_tensor("dmask", [128, 128], bf16, kind="ExternalInput")
    elif mode == 'general':
        mT_d = nc.dram_tensor("maskT", [S, S], bf16, kind="ExternalInput")
    out_d = nc.dram_tensor("out", [S, CW], bf16, kind="ExternalOutput")

    with tile.TileContext(nc) as tc, ExitStack() as ctx:
        hp = ctx.enter_context(tc.tile_pool(name="hp", bufs=2 * NDT + 4))
        xp = ctx.enter_context(tc.tile_pool(name="xp", bufs=NDT))
        wp = ctx.enter_context(tc.tile_pool(name="wp", bufs=4 * NDT))
        qkp = ctx.enter_context(tc.tile_pool(name="qkp", bufs=2 * HPC))
        vp = ctx.enter_context(tc.tile_pool(name="vp", bufs=NST))
        cst = ctx.enter_context(tc.tile_pool(name="cst", bufs=1))
        ptp = ctx.enter_context(tc.tile_pool(name="ptp", bufs=4))
        rp = ctx.enter_context(tc.tile_pool(name="rp", bufs=2))
        op = ctx.enter_context(tc.tile_pool(name="op", bufs=3))
        # PSUM: ss pairs 2x[128,1024] = 4 banks, pv 2x[128,512] = 2 banks,
        # mm (proj+outproj shared) 2x[128,512] = 2 banks
        ps_ss = ctx.enter_context(tc.tile_pool(name="ps_ss", bufs=2, space="PSUM"))
        ps_pv = ctx.enter_context(tc.tile_pool(name="ps_pv", bufs=2, space="PSUM"))
        ps_mm = ctx.enter_context(tc.tile_pool(name="ps_mm", bufs=2, space="PSUM"))
        dram = ctx.enter_context(tc.tile_pool(name="dram", bufs=1, space="DRAM"))

        # ---- startup barrier: absorb host-side core-launch skew here (it
        # overlaps the DMA-bound weight load) so the first real AllGather
        # doesn't swallow it mid-pipeline, freezing kernel DMA for ~40us.
        bar_in = dram.tile([1, 128], bf16, tag="bar_in", name="bar_in")
        bar_out = dram.tile([GPC, 128], bf16, tag="bar_out", name="bar_out")
        bar_sb = cst.tile([1, 128], bf16, tag="bar", name="bar_sb")
        nc.vector.memset(bar_sb[:], 0.0)
        nc.sync.dma_start(bar_in[:], bar_sb[:])
        nc.gpsimd.collective_compute(
            "AllGather", mybir.AluOpType.bypass,
            replica_groups=REPLICA_GROUPS,
            ins=[bar_in[:].opt()], outs=[bar_out[:].opt()])

        # ---- first-needed data first: Wv + hT(chunk 0), then Wq, Wk ----
        NW = 8              # DMAs per wall (256KB each: good issue/bw balance)
        WSTEP, HSTEP = WW // NW, HW // NW
        wv_sb = wp.tile([128, WW], bf16, tag="wv", name="wv_sb", bufs=1)
        wq_sb = wp.tile([128, WW], bf16, tag="wq", name="wq_sb", bufs=1)
        wk_sb = wp.tile([128, WW], bf16, tag="wk", name="wk_sb", bufs=1)
        wo_sb = wp.tile([128, WW], bf16, tag="wo", name="wo_sb", bufs=1)
        hTc0 = hp.tile([128, HW], bf16, tag="hT", name="hT0", bufs=2)
        for i in range(NW):
            if i == 0:
                # first pieces split in half so the first matmul can start
                # ~6us earlier
                h = WSTEP // 2
                nc.sync.dma_start(wv_sb[:, 0:h], wv_d[:, 0:h])
                nc.sync.dma_start(hTc0[:, 0:h], hT_d[:, 0:h])
                nc.sync.dma_start(wv_sb[:, h:WSTEP], wv_d[:, h:WSTEP])
                nc.sync.dma_start(hTc0[:, h:HSTEP], hT_d[:, h:HSTEP])
                continue
            nc.sync.dma_start(wv_sb[:, i * WSTEP:(i + 1) * WSTEP],
                              wv_d[:, i * WSTEP:(i + 1) * WSTEP])
            nc.sync.dma_start(hTc0[:, i * HSTEP:(i + 1) * HSTEP],
                              hT_d[:, i * HSTEP:(i + 1) * HSTEP])

        # ---- constants + Q/K weights (Q first: Q-proj runs before K-proj) ----
        cq_sb = cst.tile([128, S], bf16, tag="cq", name="cq_sb")
        sq_sb = cst.tile([128, S], bf16, tag="sq", name="sq_sb")
        ck_sb = cst.tile([128, S], bf16, tag="ck", name="ck_sb")
        sk_sb = cst.tile([128, S], bf16, tag="sk", name="sk_sb")
        for i in range(NW):
            nc.sync.dma_start(wq_sb[:, i * WSTEP:(i + 1) * WSTEP],
                              wq_d[:, i * WSTEP:(i + 1) * WSTEP])
        nc.sync.dma_start(cq_sb[:], cq_d[:])
        nc.sync.dma_start(sq_sb[:], sq_d[:])
        for i in range(NW):
            nc.sync.dma_start(wk_sb[:, i * WSTEP:(i + 1) * WSTEP],
                              wk_d[:, i * WSTEP:(i + 1) * WSTEP])
        nc.sync.dma_start(ck_sb[:], ck_d[:])
        nc.sync.dma_start(sk_sb[:], sk_d[:])
        if use_bias:
            bq_sb = cst.tile([128, HPC], f32, tag="bq", name="bq_sb")
            bk_sb = cst.tile([128, HPC], f32, tag="bk", name="bk_sb")
            bv_sb = cst.tile([1, CW], f32, tag="bv", name="bv_sb")
            bo_sb = cst.tile([1, CW], f32, tag="bo", name="bo_sb")
            nc.sync.dma_start(bq_sb[:], bq_d[:])
            nc.sync.dma_start(bk_sb[:], bk_d[:])
            nc.sync.dma_start(bv_sb[:], bv_d[:])
            nc.sync.dma_start(bo_sb[:], bo_d[:])
            bvb_sb = cst.tile([128, CW], f32, tag="bvb", name="bvb_sb")
            bob_sb = cst.tile([128, CW], f32, tag="bob", name="bob_sb")
            nc.gpsimd.partition_broadcast(bvb_sb[:], bv_sb[0:1, :])
            nc.gpsimd.partition_broadcast(bob_sb[:], bo_sb[0:1, :])
        ones_sb = cst.tile([128, 128], bf16, tag="ones", name="ones_sb")
        nc.vector.memset(ones_sb[:], 1.0)
        psw_sb = cst.tile([128, 128], bf16, tag="psw", name="psw_sb")
        nc.sync.dma_start(psw_sb[:], pw_d[:])
        if causal:
            tri_sb = cst.tile([128, 128], bf16, tag="tri", name="tri_sb")
            nc.sync.dma_start(tri_sb[:], dm_d[:])

        # Wo streams in behind everything else
        for i in range(NW):
            nc.sync.dma_start(wo_sb[:, i * WSTEP:(i + 1) * WSTEP],
                              wo_d[:, i * WSTEP:(i + 1) * WSTEP])

        # per-(chunk, head) K tiles: fresh tile per chunk so old-chunk reads
        # never depend on the current chunk's RoPE
        ktc = [[None] * HPC for _ in range(NQC)]
        v_sb = [None] * NST

        def rope(dst, ps, c_sb, s_sb, c, b_ap, prefix):
            """dst = rope(ps): ACT copy, then a matmul-swap (rotate rows by
            64 via a permutation stationary -- no DMA, so a collective's
            DMA-freeze can never stall the rope chain) + 3 vector ops.
            rows 0:64 = a (real), 64:128 = b (imag);
            c_sb rows = [cos;cos], s_sb rows = [-sin;+sin]."""
            csl = slice(c * QCH, (c + 1) * QCH)
            if use_bias:
                nc.scalar.activation(dst[:], ps[:], IDN, bias=b_ap)
            else:
                nc.scalar.activation(dst[:], ps[:], IDN)
            t1 = ps_pv.tile([128, QCH], f32, tag="pv", name=f"{prefix}t1")
            nc.tensor.matmul(t1[:], psw_sb[:], dst[:], start=True, stop=True)
            t2 = rp.tile([128, QCH], bf16, tag="t2", name=f"{prefix}t2")
            nc.vector.tensor_mul(t2[:], dst[:], c_sb[:, csl])
            nc.vector.tensor_mul(dst[:], t1[:], s_sb[:, csl])
            nc.vector.tensor_add(dst[:], dst[:], t2[:])

        def proj_head(w_sb, b_sb, c_sb, s_sb, dst, hTc, c, m, prefix):
            ps = ps_mm.tile([128, QCH], f32, tag="mm", name=f"{prefix}ps{m}_{c}")
            mm = None
            for dt in range(NDT):
                mm = nc.tensor.matmul(
                    ps[:], w_sb[:, dt * CW + m * 128:dt * CW + (m + 1) * 128],
                    hTc[:, dt * QCH:(dt + 1) * QCH],
                    start=(dt == 0), stop=(dt == NDT - 1))
            return mm, (dst, ps, c_sb, s_sb, c,
                        b_sb[:, m:m + 1] if use_bias else None,
                        f"{prefix}{m}_{c}")

        hTcs = {0: hTc0}

        def prefetch_hT(cn):
            if cn >= NQC or cn in hTcs:
                return
            nxt = hp.tile([128, HW], bf16, tag="hT", name=f"hT{cn}", bufs=2)
            for i in range(4):
                st = HW // 4
                nc.sync.dma_start(
                    nxt[:, i * st:(i + 1) * st],
                    hT_d[:, cn * HW + i * st:cn * HW + (i + 1) * st])
            hTcs[cn] = nxt

        def proj_block(c):
            if c == 0 or not causal:
                # causal mode prefetches later chunks during attention,
                # ahead of the AllGather data phase that freezes DMA
                prefetch_hT(c + 1)
            hTc = hTcs[c]
            # V for s-tiles 4c..4c+3
            for sti in range(4):
                st = 4 * c + sti
                ps = ps_mm.tile([128, CW], f32, tag="mm", name=f"psv{st}")
                for dt in range(NDT):
                    nc.tensor.matmul(
                        ps[:],
                        hTc[:, dt * QCH + sti * 128:dt * QCH + (sti + 1) * 128],
                        wv_sb[:, dt * CW:(dt + 1) * CW],
                        start=(dt == 0), stop=(dt == NDT - 1))
                vt = vp.tile([128, CW], bf16, tag="v", name=f"v{st}")
                if use_bias:
                    nc.vector.tensor_add(vt[:], ps[:], bvb_sb[:])
                else:
                    nc.scalar.activation(vt[:], ps[:], IDN)
                v_sb[st] = vt
            # Q and K interleaved per head so the ACT copies never pile up
            # in front of attention's exps, and head 0 is ready first.
            # Rope emission is deferred one projection so its swap-matmul
            # (which waits on the ACT copy) never stalls the PE FIFO.
            qtrc = []
            plast = None
            pend_rope = None
            for m in range(HPC):
                qt = qkp.tile([128, QCH], bf16, tag="qtc", name=f"qtc{c}_{m}",
                              bufs=HPC + 1 if causal else NQC * HPC)
                _, rq = proj_head(wq_sb, bq_sb if use_bias else None,
                                  cq_sb, sq_sb, qt, hTc, c, m, "q")
                if pend_rope is not None:
                    rope(*pend_rope)
                pend_rope = rq
                qtrc.append(qt)
                kt = qkp.tile([128, QCH], bf16, tag="kt", name=f"kt{c}_{m}",
                              bufs=NQC * HPC)
                plast, rk = proj_head(wk_sb, bk_sb if use_bias else None,
                                      ck_sb, sk_sb, kt, hTc, c, m, "k")
                rope(*pend_rope)
                pend_rope = rk
                ktc[c][m] = kt
            rope(*pend_rope)
            return qtrc, plast

        def attention_chunk(c, qtrc):
            if causal:
                prefetch_hT(c + 2)
            nk = 4 * c + 4 if causal else NST
            npair = nk // 2
            # AllGather pieces: last causal chunk split by head pairs so the
            # first collective fires mid-attention.
            split = (c == NQC - 1)
            if split:
                agins = [dram.tile([2 * 128, QCH], bf16, tag=f"agin{c}_{i}",
                                   name=f"agin{c}_{i}") for i in range(2)]
                agouts_l = [dram.tile([GPC * 2 * 128, QCH], bf16,
                                      tag=f"agout{c}_{i}",
                                      name=f"agout{c}_{i}") for i in range(2)]
            else:
                agins = [dram.tile([CW, QCH], bf16, tag=f"agin{c}_0",
                                   name=f"agin{c}_0")]
                agouts_l = [dram.tile([D, QCH], bf16, tag=f"agout{c}_0",
                                      name=f"agout{c}_0")]

            state = {}  # h -> (pv, se, so)
            pend = None  # (h, pr, pt, info)
            last_mm = [None]

            def emit_scores(h, pr):
                ss = ps_ss.tile([128, 2 * QCH], f32, tag="ss",
                                name=f"ss{c}_{h}_{pr}")
                info = []
                for j in range(2):
                    ki = 2 * pr + j
                    p = ki - 4 * c if causal else -1
                    c0 = max(0, 128 * p)
                    cc, col = divmod(ki, 4)
                    nc.tensor.matmul(
                        ss[:, j * QCH + c0:(j + 1) * QCH],
                        ktc[cc][h][:, col * 128:(col + 1) * 128],
                        qtrc[h][:, c0:], start=True, stop=True)
                    if causal and p >= 0:
                        nc.vector.tensor_add(
                            ss[:, j * QCH + c0:j * QCH + c0 + 128],
                            ss[:, j * QCH + c0:j * QCH + c0 + 128], tri_sb[:])
                    info.append((ki, c0))
                if mode == 'general':
                    mt = ptp.tile([128, 2 * QCH], bf16, tag="mt",
                                  name=f"mt{c}_{h}_{pr}", bufs=3)
                    for j in range(2):
                        ki = 2 * pr + j
                        nc.sync.dma_start(
                            mt[:, j * QCH:(j + 1) * QCH],
                            mT_d[ki * 128:(ki + 1) * 128,
                                 c * QCH:(c + 1) * QCH])
                    nc.vector.tensor_add(ss[:], ss[:], mt[:])
                pt = ptp.tile([128, 2 * QCH], bf16, tag="pt",
                              name=f"pt{c}_{h}_{pr}", bufs=4)
                e0 = info[0][1]   # skip fully-masked leading columns
                nc.scalar.activation(pt[:, e0:], ss[:, e0:], EXP)
                return pt, info

            def emit_pv(h, pr, pt, info):
                if h not in state:
                    pv = ps_pv.tile([128, QCH], f32, tag="pv", name=f"pv{c}_{h}")
                    sa = rp.tile([128, QCH], bf16, tag="sa", name=f"sa{c}_{h}",
                                 bufs=2)
                    state[h] = (pv, sa)
                pv, sa = state[h]
                for j, (ki, c0) in enumerate(info):
                    src = pt[:, j * QCH + c0:(j + 1) * QCH]
                    if ki == 0:
                        nc.vector.tensor_copy(sa[:], src)
                    else:
                        nc.vector.tensor_add(sa[:, c0:], sa[:, c0:], src)
                    last_mm[0] = nc.tensor.matmul(
                        pv[:, c0:], v_sb[ki][:, h * 128:(h + 1) * 128],
                        src, start=(ki == 0), stop=(ki == nk - 1),
                        skip_group_check=(c0 > 0))

            def emit_tail(h):
                pv, sa = state[h]
                # partition-reduce+broadcast the row sums with one all-ones
                # matmul (GpSimd partition_all_reduce measures 3.5us -- too
                # slow); the result tile squats in an ss-pool slot.
                smt = ps_ss.tile([128, 2 * QCH], f32, tag="ss",
                                 name=f"sm{c}_{h}")
                nc.tensor.matmul(smt[:, 0:QCH], ones_sb[:], sa[:],
                                 start=True, stop=True)
                recb = op.tile([128, QCH], f32, tag="recb",
                               name=f"recb{c}_{h}", bufs=2)
                nc.vector.reciprocal_approx_fast(out=recb[:], in_=smt[:, 0:QCH])
                ot = op.tile([128, QCH], bf16, tag="ot", name=f"ot{c}_{h}",
                             bufs=3)
                nc.vector.tensor_mul(ot[:], pv[:], recb[:])
                if split:
                    nc.sync.dma_start(agins[h // 2][(h % 2) * 128:
                                                    (h % 2 + 1) * 128, :], ot[:])
                    if h % 2 == 1:
                        nc.gpsimd.collective_compute(
                            "AllGather", mybir.AluOpType.bypass,
                            replica_groups=REPLICA_GROUPS,
                            ins=[agins[h // 2][:].opt()],
                            outs=[agouts_l[h // 2][:].opt()])
                else:
                    nc.sync.dma_start(agins[0][h * 128:(h + 1) * 128, :], ot[:])

            units = [(h, pr) for h in range(HPC) for pr in range(npair)]
            for h, pr in units:
                cur = (h, pr, *emit_scores(h, pr))
                if pend is not None:
                    emit_pv(pend[0], pend[1], pend[2], pend[3])
                    if pend[1] == npair - 1:
                        emit_tail(pend[0])
                pend = cur
            emit_pv(pend[0], pend[1], pend[2], pend[3])
            emit_tail(pend[0])
            if not split:
                nc.gpsimd.collective_compute(
                    "AllGather", mybir.AluOpType.bypass,
                    replica_groups=REPLICA_GROUPS,
                    ins=[agins[0][:].opt()], outs=[agouts_l[0][:].opt()])
            # pieces: list of (agout, [global d-tile per 128-row block])
            if split:
                pieces = [(agouts_l[i],
                           [4 * (j // 2) + 2 * i + (j % 2) for j in range(8)])
                          for i in range(2)]
            else:
                pieces = [(agouts_l[0], list(range(NDT)))]
            return pieces, last_mm[0]

        def prefetch_xt(qc):
            xt = []   # list per piece
            for pi, (agout, dts) in enumerate(pieces[qc]):
                tiles = []
                for j, dt in enumerate(dts):
                    t = xp.tile([128, QCH], bf16, tag="xt",
                                name=f"xt{qc}_{pi}_{j}")
                    nc.sync.dma_start(t[:], agout[j * 128:(j + 1) * 128, :])
                    tiles.append((dt, t))
                xt.append(tiles)
            return xt

        def outproj_chunk(qc, xt, after_mm=None):
            npieces = len(xt)
            nst4 = QCH // 128
            # For the split (last) chunk: run all piece-0-dependent matmuls
            # across all four output groups before any piece-1 matmul, so
            # the first AllGather's work fills the second one's latency.
            # The four concurrent PSUM groups squat in the idle ss pool.
            if npieces > 1:
                pss = [ps_ss.tile([128, 2 * QCH], f32, tag="ss",
                                  name=f"pso{qc}_{g}") for g in range(2)]
                groups = [pss[g][:, half * QCH:(half + 1) * QCH]
                          for g in (0, 1) for half in (0, 1)]
            else:
                groups = [None] * nst4
            for pi in range(npieces):
                last_piece = pi == npieces - 1
                for st4 in range(nst4):
                    if groups[st4] is None:
                        groups[st4] = ps_mm.tile([128, CW], f32, tag="mm",
                                                 name=f"pso{qc}_{st4}")[:]
                    n = len(xt[pi])
                    for i, (dt, t) in enumerate(xt[pi]):
                        mm = nc.tensor.matmul(
                            groups[st4][:], t[:, st4 * 128:(st4 + 1) * 128],
                            wo_sb[:, dt * CW:(dt + 1) * CW],
                            start=(pi == 0 and i == 0),
                            stop=(last_piece and i == n - 1))
                        if after_mm is not None:
                            # keep outproj(qc) late in PE program order: the
                            # static scheduler underestimates AllGather
                            # latency and would hoist it otherwise.
                            add_dep_helper(mm.ins, after_mm.ins, sync=False,
                                           reason="outproj ordering")
                            after_mm = None
                    if not last_piece:
                        continue
                    row = qc * QCH + st4 * 128
                    of = op.tile([128, CW], bf16, tag="of",
                                 name=f"of{qc}_{st4}", bufs=2)
                    if use_bias:
                        nc.vector.tensor_add(of[:], groups[st4][:], bob_sb[:])
                    else:
                        nc.scalar.activation(of[:], groups[st4][:], IDN)
                    nc.sync.dma_start(out_d[row:row + 128, :], of[:])

        # ---- main pipeline ----
        # outproj(c) runs two phases after its AllGather fires so even a
        # slow collective never blocks the PE queue:
        # proj0 attn0 / proj1 attn1 / proj2 op0 attn2 / proj3 op1 attn3 /
        # op2 op3
        pieces, last_mms, xts = {}, {}, {}
        if causal:
            for c in range(NQC):
                qtrc, plast = proj_block(c)
                if c >= 2:
                    outproj_chunk(c - 2, xts[c - 2], plast)
                pieces[c], last_mms[c] = attention_chunk(c, qtrc)
                if c >= 1:
                    # xt loads for chunk c-1 issue here: after this chunk's
                    # agin writes (no gating risk) and while no AllGather
                    # data phase is freezing the DMA queues
                    xts[c - 1] = prefetch_xt(c - 1)
            outproj_chunk(NQC - 2, xts[NQC - 2], last_mms[NQC - 1])
            xts[NQC - 1] = prefetch_xt(NQC - 1)
            outproj_chunk(NQC - 1, xts[NQC - 1], last_mms[NQC - 1])
        else:
            # non-causal: attention(c) needs the FULL K/V, so project
            # everything first, then run the attention/AG/outproj pipeline
            qtrcs = [proj_block(c)[0] for c in range(NQC)]
            for c in range(NQC):
                pieces[c], last_mms[c] = attention_chunk(c, qtrcs[c])
                xts[c] = prefetch_xt(c)
                if c > 0:
                    outproj_chunk(c - 1, xts[c - 1], last_mms[c])
            outproj_chunk(NQC - 1, xts[NQC - 1], last_mms[NQC - 1])

    nc.compile()
    return nc


def _get_built(mode: str, use_bias: bool):
    key = (mode, use_bias)
    if key not in _built:
        _built[key] = _build(mode, use_bias)
    return _built[key]


def _prep_inputs(inputs, mode, use_bias):
    hs = np.asarray(inputs["hidden_states"], np.float32)
    fc = np.asarray(inputs["freqs_cis"], np.float32)
    Wq = np.asarray(inputs["Wq"], np.float32)
    Wk = np.asarray(inputs["Wk"], np.float32)
    Wv = np.asarray(inputs["Wv"], np.float32)
    Wo = np.asarray(inputs["Wo"], np.float32)
    bq = np.asarray(inputs["bq"], np.float32)
    bk = np.asarray(inputs["bk"], np.float32)
    bv = np.asarray(inputs["bv"], np.float32)
    bo = np.asarray(inputs["bo"], np.float32)

    # de-interleave permutation per 128-row head block: [0,2,..,126, 1,3,..,127]
    perm1 = np.concatenate([np.arange(0, DH, 2), np.arange(1, DH, 2)])
    permC = (np.arange(CW) // DH) * DH  # head base offsets
    perm = permC + perm1[np.arange(CW) % DH]

    scale = 1.0 / math.sqrt(DH)
    cos = np.concatenate([fc[:, :, 0].T, fc[:, :, 0].T])  # [128, S], dup halves
    sinT = fc[:, :, 1].T
    sin2 = np.concatenate([-sinT, sinT])                  # sign-folded
    cq = np.ascontiguousarray(cos * scale).astype(BF)
    sq2 = np.ascontiguousarray(sin2 * scale).astype(BF)
    ck = np.ascontiguousarray(cos).astype(BF)
    sk2 = np.ascontiguousarray(sin2).astype(BF)

    pswap = np.zeros((128, 128), np.float32)
    pswap[(np.arange(128) + 64) % 128, np.arange(128)] = 1.0
    pswap = pswap.astype(BF)

    if mode == 'causal':
        tri = np.where(np.arange(128)[:, None] > np.arange(128)[None, :],
                       np.float32(NEG), np.float32(0.0)).astype(BF)
    elif mode == 'general':
        maskT = np.ascontiguousarray(
            np.asarray(inputs["mask"], np.float32)[0, 0].T).astype(BF)

    def pack_w(wT):
        # [D, CW] -> [128, NDT*CW] wall (d-tile rows side by side)
        return np.ascontiguousarray(
            np.concatenate([wT[dt * 128:(dt + 1) * 128, :]
                            for dt in range(NDT)], axis=1)).astype(BF)

    def pack_h(hsT):
        # [D, S] -> [128, NQC*NDT*QCH] wall, chunk-major then d-tile
        segs = [hsT[dt * 128:(dt + 1) * 128, c * QCH:(c + 1) * QCH]
                for c in range(NQC) for dt in range(NDT)]
        return np.ascontiguousarray(np.concatenate(segs, axis=1)).astype(BF)

    hTb = [pack_h(hs[b].T) for b in range(B)]

    in_maps = []
    for c in range(NCORES):
        b, hg = divmod(c, GPC)
        sl = slice(CW * hg, CW * (hg + 1))
        wq_s = Wq[sl][perm]
        wk_s = Wk[sl][perm]
        m = {
            "hiddenT": hTb[b],
            "wqT": pack_w(wq_s.T),
            "wkT": pack_w(wk_s.T),
            "wvT": pack_w(Wv[sl].T),
            "woT": pack_w(Wo[sl].T),
            "cq": cq, "sq2": sq2, "ck": ck, "sk2": sk2, "pswap": pswap,
        }
        if use_bias:
            m["bqp"] = np.ascontiguousarray(
                bq[sl][perm].reshape(HPC, 128).T).astype(np.float32)
            m["bkp"] = np.ascontiguousarray(
                bk[sl][perm].reshape(HPC, 128).T).astype(np.float32)
            m["bv2"] = bv[sl].reshape(1, CW).astype(np.float32)
            m["bo2"] = bo[sl].reshape(1, CW).astype(np.float32)
        if mode == 'causal':
            m["dmask"] = tri
        elif mode == 'general':
            m["maskT"] = maskT
        in_maps.append(m)
    return in_maps


def _mask_mode(mask):
    mask = np.asarray(mask, np.float32)
    if mask.shape != (1, 1, S, S):
        return 'general'
    m = mask[0, 0]
    if not np.any(m):
        return 'zero'
    expect = np.triu(np.full((S, S), np.float32(NEG)), k=1)
    if np.array_equal(m, expect):
        return 'causal'
    return 'general'


def run_on_cores(inputs, trace=False):
    """Compile+run; returns BassKernelResults."""
    from concourse.bass_utils import run_bass_kernel_spmd
    mode = _mask_mode(inputs["mask"])
    use_bias = any(
        np.any(np.asarray(inputs[k])) for k in ("bq", "bk", "bv", "bo"))
    nc = _get_built(mode, use_bias)
    in_maps = _prep_inputs(inputs, mode, use_bias)
    r = run_bass_kernel_spmd(nc, in_maps, list(range(NCORES)), trace=trace)
    return r


def kernel(**inputs) -> np.ndarray:
    r = run_on_cores(inputs)
    out = np.empty((B, S, D), np.float32)
    for c in range(NCORES):
        b, hg = divmod(c, GPC)
        out[b, :, CW * hg:CW * (hg + 1)] = \
            np.asarray(r.results[c]["out"]).astype(np.float32)
    return out


# revision 48
# speedup vs baseline: 1.0623x; 1.0275x over previous
"""Self-contained Trainium2 Bass kernel: 16-head attention with RoPE (B=2, S=2048, D=2048).

Sharding: 8 cores = 2 (batch) x 4 (head groups of 4 heads / 512 cols).
Per core: QKV projections for its head group -> RoPE -> causal attention ->
AllGather of attention outputs (X^T) within the 4-core batch group ->
column-sharded output projection. Host assembles by concatenation only.

Pipeline over 4 q-chunks of 512:
  chunk c: V(s-tiles 4c..4c+3), per head {Q(c)+RoPE, K(c)+RoPE} interleaved,
           attention(c) [causal: k-tiles 0..4c+3], AllGather(c),
           outproj scheduled late so collectives overlap compute.

Dataflow is fully "transposed" so no on-chip transposes are needed:
  hiddenT [d, s] (host-pretransposed, bf16), streamed per chunk
  QT/KT   [dh, s] per head  (projection emits head-dim-major directly;
          K is stored in per-chunk tiles to avoid false RoPE deps)
  S^T     [k, q] scores, two k-tiles paired in one [128,1024] PSUM tile
  P^T     [k, q] = exp(S^T + mask^T), one ACT op per pair
  row-sums accumulated in SBUF split across Vector/GpSimd engines,
  partition-reduced+broadcast with one all-ones matmul
  O^T     [dh, q] = V^T @ P^T          (lhsT = V natural [s, dh])
  X^T     AllGather on first axis (last chunk split by head pairs)
  out     [s, oc] (lhsT = X^T block, rhs = WoT)
RoPE de-interleave is folded into a host-side row permutation of Wq/Wk;
RoPE itself is 3 full-height vector ops with sign-folded sin tables.
1/sqrt(DH) is folded into the Q rope tables.
"""

import math
from contextlib import ExitStack

import numpy as np
import ml_dtypes

B, S, D, H, DH = 2, 2048, 2048, 16, 128
NCORES = 8
GPC = 4            # cores per tensor-parallel group
HPC = H // GPC     # heads per core (4)
CW = HPC * DH      # 512 columns per core
NEG = -1e9
BF = ml_dtypes.bfloat16
QCH = 512          # q-chunk (moving free dim)
NQC = S // QCH     # 4
NDT = D // 128     # 16 d-tiles
NST = S // 128     # 16 s-tiles

REPLICA_GROUPS = [[0, 1, 2, 3], [4, 5, 6, 7]]

_built = {}


def _build(mode: str, use_bias: bool):
    # mode: 'causal' (standard causal mask), 'zero' (mask is all zeros),
    #       'general' (arbitrary additive mask, streamed from DRAM)
    import concourse.bass as bass
    import concourse.tile as tile
    from concourse import bacc, bass_isa, mybir
    from concourse.tile_rust import add_dep_helper

    causal = mode == 'causal'

    f32, bf16 = mybir.dt.float32, mybir.dt.bfloat16
    EXP = mybir.ActivationFunctionType.Exp
    IDN = mybir.ActivationFunctionType.Identity

    nc = bacc.Bacc("TRN2", target_bir_lowering=False, debug=False,
                   num_devices=NCORES)

    # weights and hidden states arrive host-packed as [128, tiles*cols] walls
    # so each loads with a handful of large DMAs (the SP sequencer issues
    # DMA triggers at ~650ns each -- many small DMAs throttle startup)
    WW = NDT * CW      # 8192 wall columns per weight matrix
    HW = NDT * QCH     # 8192 wall columns per hidden-state chunk
    hT_d = nc.dram_tensor("hiddenT", [128, NQC * HW], bf16, kind="ExternalInput")
    wq_d = nc.dram_tensor("wqT", [128, WW], bf16, kind="ExternalInput")
    wk_d = nc.dram_tensor("wkT", [128, WW], bf16, kind="ExternalInput")
    wv_d = nc.dram_tensor("wvT", [128, WW], bf16, kind="ExternalInput")
    wo_d = nc.dram_tensor("woT", [128, WW], bf16, kind="ExternalInput")
    cq_d = nc.dram_tensor("cq", [128, S], bf16, kind="ExternalInput")
    sq_d = nc.dram_tensor("sq2", [128, S], bf16, kind="ExternalInput")
    ck_d = nc.dram_tensor("ck", [128, S], bf16, kind="ExternalInput")
    sk_d = nc.dram_tensor("sk2", [128, S], bf16, kind="ExternalInput")
    pw_d = nc.dram_tensor("pswap", [128, 128], bf16, kind="ExternalInput")
    if use_bias:
        bq_d = nc.dram_tensor("bqp", [128, HPC], f32, kind="ExternalInput")
        bk_d = nc.dram_tensor("bkp", [128, HPC], f32, kind="ExternalInput")
        bv_d = nc.dram_tensor("bv2", [1, CW], f32, kind="ExternalInput")
        bo_d = nc.dram_tensor("bo2", [1, CW], f32, kind="ExternalInput")
    if causal:
        dm_d = nc.dram_tensor("dmask", [128, 128], bf16, kind="ExternalInput")
    elif mode == 'general':
        mT_d = nc.dram_tensor("maskT", [S, S], bf16, kind="ExternalInput")
    out_d = nc.dram_tensor("out", [S, CW], bf16, kind="ExternalOutput")

    with tile.TileContext(nc) as tc, ExitStack() as ctx:
        hp = ctx.enter_context(tc.tile_pool(name="hp", bufs=2 * NDT + 4))
        xp = ctx.enter_context(tc.tile_pool(name="xp", bufs=NDT))
        wp = ctx.enter_context(tc.tile_pool(name="wp", bufs=4 * NDT))
        qkp = ctx.enter_context(tc.tile_pool(name="qkp", bufs=2 * HPC))
        vp = ctx.enter_context(tc.tile_pool(name="vp", bufs=NST))
        cst = ctx.enter_context(tc.tile_pool(name="cst", bufs=1))
        ptp = ctx.enter_context(tc.tile_pool(name="ptp", bufs=4))
        rp = ctx.enter_context(tc.tile_pool(name="rp", bufs=2))
        op = ctx.enter_context(tc.tile_pool(name="op", bufs=3))
        # PSUM: ss pairs 2x[128,1024] = 4 banks, pv 2x[128,512] = 2 banks,
        # mm (proj+outproj shared) 2x[128,512] = 2 banks
        ps_ss = ctx.enter_context(tc.tile_pool(name="ps_ss", bufs=2, space="PSUM"))
        ps_pv = ctx.enter_context(tc.tile_pool(name="ps_pv", bufs=2, space="PSUM"))
        ps_mm = ctx.enter_context(tc.tile_pool(name="ps_mm", bufs=2, space="PSUM"))
        dram = ctx.enter_context(tc.tile_pool(name="dram", bufs=1, space="DRAM"))

        # ---- startup barrier: absorb host-side core-launch skew here (it
        # overlaps the DMA-bound weight load) so the first real AllGather
        # doesn't swallow it mid-pipeline, freezing kernel DMA for ~40us.
        bar_in = dram.tile([1, 128], bf16, tag="bar_in", name="bar_in")
        bar_out = dram.tile([GPC, 128], bf16, tag="bar_out", name="bar_out")
        bar_sb = cst.tile([1, 128], bf16, tag="bar", name="bar_sb")
        nc.vector.memset(bar_sb[:], 0.0)
        nc.sync.dma_start(bar_in[:], bar_sb[:])
        nc.gpsimd.collective_compute(
            "AllGather", mybir.AluOpType.bypass,
            replica_groups=REPLICA_GROUPS,
            ins=[bar_in[:].opt()], outs=[bar_out[:].opt()])

        # ---- first-needed data first: Wv + hT(chunk 0), then Wq, Wk ----
        NW = 8              # DMAs per wall (256KB each: good issue/bw balance)
        WSTEP, HSTEP = WW // NW, HW // NW
        wv_sb = wp.tile([128, WW], bf16, tag="wv", name="wv_sb", bufs=1)
        wq_sb = wp.tile([128, WW], bf16, tag="wq", name="wq_sb", bufs=1)
        wk_sb = wp.tile([128, WW], bf16, tag="wk", name="wk_sb", bufs=1)
        wo_sb = wp.tile([128, WW], bf16, tag="wo", name="wo_sb", bufs=1)
        hTc0 = hp.tile([128, HW], bf16, tag="hT", name="hT0", bufs=2)
        for i in range(NW):
            if i == 0:
                # first pieces split in half so the first matmul can start
                # ~6us earlier
                h = WSTEP // 2
                nc.sync.dma_start(wv_sb[:, 0:h], wv_d[:, 0:h])
                nc.sync.dma_start(hTc0[:, 0:h], hT_d[:, 0:h])
                nc.sync.dma_start(wv_sb[:, h:WSTEP], wv_d[:, h:WSTEP])
                nc.sync.dma_start(hTc0[:, h:HSTEP], hT_d[:, h:HSTEP])
                continue
            nc.sync.dma_start(wv_sb[:, i * WSTEP:(i + 1) * WSTEP],
                              wv_d[:, i * WSTEP:(i + 1) * WSTEP])
            nc.sync.dma_start(hTc0[:, i * HSTEP:(i + 1) * HSTEP],
                              hT_d[:, i * HSTEP:(i + 1) * HSTEP])

        # ---- constants + Q/K weights (Q first: Q-proj runs before K-proj) ----
        cq_sb = cst.tile([128, S], bf16, tag="cq", name="cq_sb")
        sq_sb = cst.tile([128, S], bf16, tag="sq", name="sq_sb")
        ck_sb = cst.tile([128, S], bf16, tag="ck", name="ck_sb")
        sk_sb = cst.tile([128, S], bf16, tag="sk", name="sk_sb")
        for i in range(NW):
            nc.sync.dma_start(wq_sb[:, i * WSTEP:(i + 1) * WSTEP],
                              wq_d[:, i * WSTEP:(i + 1) * WSTEP])
        nc.sync.dma_start(cq_sb[:], cq_d[:])
        nc.sync.dma_start(sq_sb[:], sq_d[:])
        for i in range(NW):
            nc.sync.dma_start(wk_sb[:, i * WSTEP:(i + 1) * WSTEP],
                              wk_d[:, i * WSTEP:(i + 1) * WSTEP])
        nc.sync.dma_start(ck_sb[:], ck_d[:])
        nc.sync.dma_start(sk_sb[:], sk_d[:])
        if use_bias:
            bq_sb = cst.tile([128, HPC], f32, tag="bq", name="bq_sb")
            bk_sb = cst.tile([128, HPC], f32, tag="bk", name="bk_sb")
            bv_sb = cst.tile([1, CW], f32, tag="bv", name="bv_sb")
            bo_sb = cst.tile([1, CW], f32, tag="bo", name="bo_sb")
            nc.sync.dma_start(bq_sb[:], bq_d[:])
            nc.sync.dma_start(bk_sb[:], bk_d[:])
            nc.sync.dma_start(bv_sb[:], bv_d[:])
            nc.sync.dma_start(bo_sb[:], bo_d[:])
            bvb_sb = cst.tile([128, CW], f32, tag="bvb", name="bvb_sb")
            bob_sb = cst.tile([128, CW], f32, tag="bob", name="bob_sb")
            nc.gpsimd.partition_broadcast(bvb_sb[:], bv_sb[0:1, :])
            nc.gpsimd.partition_broadcast(bob_sb[:], bo_sb[0:1, :])
        ones_sb = cst.tile([128, 128], bf16, tag="ones", name="ones_sb")
        nc.vector.memset(ones_sb[:], 1.0)
        psw_sb = cst.tile([128, 128], bf16, tag="psw", name="psw_sb")
        nc.sync.dma_start(psw_sb[:], pw_d[:])
        if causal:
            tri_sb = cst.tile([128, 128], bf16, tag="tri", name="tri_sb")
            nc.sync.dma_start(tri_sb[:], dm_d[:])

        # Wo streams in behind everything else
        for i in range(NW):
            nc.sync.dma_start(wo_sb[:, i * WSTEP:(i + 1) * WSTEP],
                              wo_d[:, i * WSTEP:(i + 1) * WSTEP])

        # per-(chunk, head) K tiles: fresh tile per chunk so old-chunk reads
        # never depend on the current chunk's RoPE
        ktc = [[None] * HPC for _ in range(NQC)]
        v_sb = [None] * NST

        def rope(dst, ps, c_sb, s_sb, c, b_ap, prefix):
            """dst = rope(ps): ACT copy, then a matmul-swap (rotate rows by
            64 via a permutation stationary -- no DMA, so a collective's
            DMA-freeze can never stall the rope chain) + 3 vector ops.
            rows 0:64 = a (real), 64:128 = b (imag);
            c_sb rows = [cos;cos], s_sb rows = [-sin;+sin]."""
            csl = slice(c * QCH, (c + 1) * QCH)
            if use_bias:
                nc.scalar.activation(dst[:], ps[:], IDN, bias=b_ap)
            else:
                nc.scalar.activation(dst[:], ps[:], IDN)
            t1 = ps_pv.tile([128, QCH], f32, tag="pv", name=f"{prefix}t1")
            nc.tensor.matmul(t1[:], psw_sb[:], dst[:], start=True, stop=True)
            t2 = rp.tile([128, QCH], bf16, tag="t2", name=f"{prefix}t2")
            nc.vector.tensor_mul(t2[:], dst[:], c_sb[:, csl])
            nc.vector.tensor_mul(dst[:], t1[:], s_sb[:, csl])
            nc.vector.tensor_add(dst[:], dst[:], t2[:])

        def proj_head(w_sb, b_sb, c_sb, s_sb, dst, hTc, c, m, prefix):
            ps = ps_mm.tile([128, QCH], f32, tag="mm", name=f"{prefix}ps{m}_{c}")
            mm = None
            for dt in range(NDT):
                mm = nc.tensor.matmul(
                    ps[:], w_sb[:, dt * CW + m * 128:dt * CW + (m + 1) * 128],
                    hTc[:, dt * QCH:(dt + 1) * QCH],
                    start=(dt == 0), stop=(dt == NDT - 1))
            return mm, (dst, ps, c_sb, s_sb, c,
                        b_sb[:, m:m + 1] if use_bias else None,
                        f"{prefix}{m}_{c}")

        hTcs = {0: hTc0}

        def prefetch_hT(cn):
            if cn >= NQC or cn in hTcs:
                return
            nxt = hp.tile([128, HW], bf16, tag="hT", name=f"hT{cn}", bufs=2)
            for i in range(4):
                st = HW // 4
                nc.sync.dma_start(
                    nxt[:, i * st:(i + 1) * st],
                    hT_d[:, cn * HW + i * st:cn * HW + (i + 1) * st])
            hTcs[cn] = nxt

        def proj_block(c):
            if c == 0 or not causal:
                # causal mode prefetches later chunks during attention,
                # ahead of the AllGather data phase that freezes DMA
                prefetch_hT(c + 1)
            hTc = hTcs[c]
            # V for s-tiles 4c..4c+3
            for sti in range(4):
                st = 4 * c + sti
                ps = ps_mm.tile([128, CW], f32, tag="mm", name=f"psv{st}")
                for dt in range(NDT):
                    nc.tensor.matmul(
                        ps[:],
                        hTc[:, dt * QCH + sti * 128:dt * QCH + (sti + 1) * 128],
                        wv_sb[:, dt * CW:(dt + 1) * CW],
                        start=(dt == 0), stop=(dt == NDT - 1))
                vt = vp.tile([128, CW], bf16, tag="v", name=f"v{st}")
                if use_bias:
                    nc.vector.tensor_add(vt[:], ps[:], bvb_sb[:])
                else:
                    nc.scalar.activation(vt[:], ps[:], IDN)
                v_sb[st] = vt
            # Q and K interleaved per head so the ACT copies never pile up
            # in front of attention's exps, and head 0 is ready first.
            # Rope emission is deferred one projection so its swap-matmul
            # (which waits on the ACT copy) never stalls the PE FIFO.
            qtrc = []
            plast = None
            pend_rope = None
            for m in range(HPC):
                qt = qkp.tile([128, QCH], bf16, tag="qtc", name=f"qtc{c}_{m}",
                              bufs=HPC + 1 if causal else NQC * HPC)
                _, rq = proj_head(wq_sb, bq_sb if use_bias else None,
                                  cq_sb, sq_sb, qt, hTc, c, m, "q")
                if pend_rope is not None:
                    rope(*pend_rope)
                pend_rope = rq
                qtrc.append(qt)
                kt = qkp.tile([128, QCH], bf16, tag="kt", name=f"kt{c}_{m}",
                              bufs=NQC * HPC)
                plast, rk = proj_head(wk_sb, bk_sb if use_bias else None,
                                      ck_sb, sk_sb, kt, hTc, c, m, "k")
                rope(*pend_rope)
                pend_rope = rk
                ktc[c][m] = kt
            rope(*pend_rope)
            return qtrc, plast

        def attention_chunk(c, qtrc, fill=None):
            if causal:
                prefetch_hT(c + 2)
            nk = 4 * c + 4 if causal else NST
            npair = nk // 2

            def draw_fill(n):
                nonlocal fill
                if fill is None:
                    return
                try:
                    for _ in range(n):
                        next(fill)
                except StopIteration:
                    fill = None
            # AllGather pieces: last causal chunk split by head pairs so the
            # first collective fires mid-attention.
            split = (c == NQC - 1)
            if split:
                agins = [dram.tile([2 * 128, QCH], bf16, tag=f"agin{c}_{i}",
                                   name=f"agin{c}_{i}") for i in range(2)]
                agouts_l = [dram.tile([GPC * 2 * 128, QCH], bf16,
                                      tag=f"agout{c}_{i}",
                                      name=f"agout{c}_{i}") for i in range(2)]
            else:
                agins = [dram.tile([CW, QCH], bf16, tag=f"agin{c}_0",
                                   name=f"agin{c}_0")]
                agouts_l = [dram.tile([D, QCH], bf16, tag=f"agout{c}_0",
                                      name=f"agout{c}_0")]

            state = {}  # h -> (pv, se, so)
            pend = None  # (h, pr, pt, info)
            last_mm = [None]

            def emit_scores(h, pr):
                ss = ps_ss.tile([128, 2 * QCH], f32, tag="ss",
                                name=f"ss{c}_{h}_{pr}")
                info = []
                for j in range(2):
                    ki = 2 * pr + j
                    p = ki - 4 * c if causal else -1
                    c0 = max(0, 128 * p)
                    cc, col = divmod(ki, 4)
                    nc.tensor.matmul(
                        ss[:, j * QCH + c0:(j + 1) * QCH],
                        ktc[cc][h][:, col * 128:(col + 1) * 128],
                        qtrc[h][:, c0:], start=True, stop=True)
                    if causal and p >= 0:
                        nc.vector.tensor_add(
                            ss[:, j * QCH + c0:j * QCH + c0 + 128],
                            ss[:, j * QCH + c0:j * QCH + c0 + 128], tri_sb[:])
                    info.append((ki, c0))
                if mode == 'general':
                    mt = ptp.tile([128, 2 * QCH], bf16, tag="mt",
                                  name=f"mt{c}_{h}_{pr}", bufs=3)
                    for j in range(2):
                        ki = 2 * pr + j
                        nc.sync.dma_start(
                            mt[:, j * QCH:(j + 1) * QCH],
                            mT_d[ki * 128:(ki + 1) * 128,
                                 c * QCH:(c + 1) * QCH])
                    nc.vector.tensor_add(ss[:], ss[:], mt[:])
                pt = ptp.tile([128, 2 * QCH], bf16, tag="pt",
                              name=f"pt{c}_{h}_{pr}", bufs=4)
                e0 = info[0][1]   # skip fully-masked leading columns
                nc.scalar.activation(pt[:, e0:], ss[:, e0:], EXP)
                return pt, info

            def emit_pv(h, pr, pt, info):
                if h not in state:
                    pv = ps_pv.tile([128, QCH], f32, tag="pv", name=f"pv{c}_{h}")
                    sa = rp.tile([128, QCH], bf16, tag="sa", name=f"sa{c}_{h}",
                                 bufs=2)
                    state[h] = (pv, sa)
                pv, sa = state[h]
                for j, (ki, c0) in enumerate(info):
                    src = pt[:, j * QCH + c0:(j + 1) * QCH]
                    if ki == 0:
                        nc.vector.tensor_copy(sa[:], src)
                    else:
                        nc.vector.tensor_add(sa[:, c0:], sa[:, c0:], src)
                    last_mm[0] = nc.tensor.matmul(
                        pv[:, c0:], v_sb[ki][:, h * 128:(h + 1) * 128],
                        src, start=(ki == 0), stop=(ki == nk - 1),
                        skip_group_check=(c0 > 0))

            def emit_tail(h):
                pv, sa = state[h]
                # partition-reduce+broadcast the row sums with one all-ones
                # matmul (GpSimd partition_all_reduce measures 3.5us -- too
                # slow); the result tile squats in an ss-pool slot.
                smt = ps_ss.tile([128, 2 * QCH], f32, tag="ss",
                                 name=f"sm{c}_{h}")
                nc.tensor.matmul(smt[:, 0:QCH], ones_sb[:], sa[:],
                                 start=True, stop=True)
                recb = op.tile([128, QCH], f32, tag="recb",
                               name=f"recb{c}_{h}", bufs=2)
                nc.vector.reciprocal_approx_fast(out=recb[:], in_=smt[:, 0:QCH])
                ot = op.tile([128, QCH], bf16, tag="ot", name=f"ot{c}_{h}",
                             bufs=3)
                nc.vector.tensor_mul(ot[:], pv[:], recb[:])
                if split:
                    nc.sync.dma_start(agins[h // 2][(h % 2) * 128:
                                                    (h % 2 + 1) * 128, :], ot[:])
                    if h % 2 == 1:
                        nc.gpsimd.collective_compute(
                            "AllGather", mybir.AluOpType.bypass,
                            replica_groups=REPLICA_GROUPS,
                            ins=[agins[h // 2][:].opt()],
                            outs=[agouts_l[h // 2][:].opt()])
                else:
                    nc.sync.dma_start(agins[0][h * 128:(h + 1) * 128, :], ot[:])

            units = [(h, pr) for h in range(HPC) for pr in range(npair)]
            for h, pr in units:
                cur = (h, pr, *emit_scores(h, pr))
                if pend is not None:
                    emit_pv(pend[0], pend[1], pend[2], pend[3])
                    if pend[1] == npair - 1:
                        emit_tail(pend[0])
                draw_fill(2)
                pend = cur
            emit_pv(pend[0], pend[1], pend[2], pend[3])
            emit_tail(pend[0])
            draw_fill(1 << 20)   # drain whatever outproj work remains
            if not split:
                nc.gpsimd.collective_compute(
                    "AllGather", mybir.AluOpType.bypass,
                    replica_groups=REPLICA_GROUPS,
                    ins=[agins[0][:].opt()], outs=[agouts_l[0][:].opt()])
            # pieces: list of (agout, [global d-tile per 128-row block])
            if split:
                pieces = [(agouts_l[i],
                           [4 * (j // 2) + 2 * i + (j % 2) for j in range(8)])
                          for i in range(2)]
            else:
                pieces = [(agouts_l[0], list(range(NDT)))]
            return pieces, last_mm[0]

        def prefetch_xt(qc):
            xt = []   # list per piece
            for pi, (agout, dts) in enumerate(pieces[qc]):
                tiles = []
                for j, dt in enumerate(dts):
                    t = xp.tile([128, QCH], bf16, tag="xt",
                                name=f"xt{qc}_{pi}_{j}")
                    nc.sync.dma_start(t[:], agout[j * 128:(j + 1) * 128, :])
                    tiles.append((dt, t))
                xt.append(tiles)
            return xt

        def outproj_steps(qc, xt, after_mm=None, of_on_vector=False):
            """Generator emitting outproj one matmul per step, so the
            caller can interleave it into an ACT-paced attention phase."""
            npieces = len(xt)
            nst4 = QCH // 128
            # For the split (last) chunk: run all piece-0-dependent matmuls
            # across all four output groups before any piece-1 matmul, so
            # the first AllGather's work fills the second one's latency.
            # The four concurrent PSUM groups squat in the idle ss pool.
            if npieces > 1:
                pss = [ps_ss.tile([128, 2 * QCH], f32, tag="ss",
                                  name=f"pso{qc}_{g}") for g in range(2)]
                groups = [pss[g][:, half * QCH:(half + 1) * QCH]
                          for g in (0, 1) for half in (0, 1)]
            else:
                groups = [None] * nst4
            for pi in range(npieces):
                last_piece = pi == npieces - 1
                for st4 in range(nst4):
                    if groups[st4] is None:
                        groups[st4] = ps_mm.tile([128, CW], f32, tag="mm",
                                                 name=f"pso{qc}_{st4}")[:]
                    n = len(xt[pi])
                    for i, (dt, t) in enumerate(xt[pi]):
                        mm = nc.tensor.matmul(
                            groups[st4][:], t[:, st4 * 128:(st4 + 1) * 128],
                            wo_sb[:, dt * CW:(dt + 1) * CW],
                            start=(pi == 0 and i == 0),
                            stop=(last_piece and i == n - 1))
                        if after_mm is not None:
                            # keep outproj(qc) late in PE program order: the
                            # static scheduler underestimates AllGather
                            # latency and would hoist it otherwise.
                            add_dep_helper(mm.ins, after_mm.ins, sync=False,
                                           reason="outproj ordering")
                            after_mm = None
                        yield
                    if not last_piece:
                        continue
                    row = qc * QCH + st4 * 128
                    of = op.tile([128, CW], bf16, tag="of",
                                 name=f"of{qc}_{st4}", bufs=2)
                    if use_bias:
                        nc.vector.tensor_add(of[:], groups[st4][:], bob_sb[:])
                    elif of_on_vector:
                        # during attention ACT is the pacer; copy on DVE
                        nc.vector.tensor_copy(of[:], groups[st4][:])
                    else:
                        nc.scalar.activation(of[:], groups[st4][:], IDN)
                    nc.sync.dma_start(out_d[row:row + 128, :], of[:])
                    yield

        def outproj_chunk(qc, xt, after_mm=None):
            for _ in outproj_steps(qc, xt, after_mm):
                pass

        # ---- main pipeline ----
        # outproj(c) runs two phases after its AllGather fires so even a
        # slow collective never blocks the PE queue:
        # proj0 attn0 / proj1 attn1 / proj2 op0 attn2 / proj3 op1 attn3 /
        # op2 op3
        pieces, last_mms, xts = {}, {}, {}
        if causal:
            for c in range(NQC):
                qtrc, plast = proj_block(c)
                # outproj(c-2) interleaves into attention(c): attention is
                # ACT-paced (exp), leaving PE slack the outproj matmuls fill
                fill = outproj_steps(c - 2, xts[c - 2], plast,
                                     of_on_vector=True) if c >= 2 else None
                pieces[c], last_mms[c] = attention_chunk(c, qtrc, fill)
                if c >= 1:
                    # xt loads for chunk c-1 issue here: after this chunk's
                    # agin writes (no gating risk) and while no AllGather
                    # data phase is freezing the DMA queues
                    xts[c - 1] = prefetch_xt(c - 1)
            outproj_chunk(NQC - 2, xts[NQC - 2], last_mms[NQC - 1])
            xts[NQC - 1] = prefetch_xt(NQC - 1)
            outproj_chunk(NQC - 1, xts[NQC - 1], last_mms[NQC - 1])
        else:
            # non-causal: attention(c) needs the FULL K/V, so project
            # everything first, then run the attention/AG/outproj pipeline
            qtrcs = [proj_block(c)[0] for c in range(NQC)]
            for c in range(NQC):
                pieces[c], last_mms[c] = attention_chunk(c, qtrcs[c])
                xts[c] = prefetch_xt(c)
                if c > 0:
                    outproj_chunk(c - 1, xts[c - 1], last_mms[c])
            outproj_chunk(NQC - 1, xts[NQC - 1], last_mms[NQC - 1])

    nc.compile()
    return nc


def _get_built(mode: str, use_bias: bool):
    key = (mode, use_bias)
    if key not in _built:
        _built[key] = _build(mode, use_bias)
    return _built[key]


def _prep_inputs(inputs, mode, use_bias):
    hs = np.asarray(inputs["hidden_states"], np.float32)
    fc = np.asarray(inputs["freqs_cis"], np.float32)
    Wq = np.asarray(inputs["Wq"], np.float32)
    Wk = np.asarray(inputs["Wk"], np.float32)
    Wv = np.asarray(inputs["Wv"], np.float32)
    Wo = np.asarray(inputs["Wo"], np.float32)
    bq = np.asarray(inputs["bq"], np.float32)
    bk = np.asarray(inputs["bk"], np.float32)
    bv = np.asarray(inputs["bv"], np.float32)
    bo = np.asarray(inputs["bo"], np.float32)

    # de-interleave permutation per 128-row head block: [0,2,..,126, 1,3,..,127]
    perm1 = np.concatenate([np.arange(0, DH, 2), np.arange(1, DH, 2)])
    permC = (np.arange(CW) // DH) * DH  # head base offsets
    perm = permC + perm1[np.arange(CW) % DH]

    scale = 1.0 / math.sqrt(DH)
    cos = np.concatenate([fc[:, :, 0].T, fc[:, :, 0].T])  # [128, S], dup halves
    sinT = fc[:, :, 1].T
    sin2 = np.concatenate([-sinT, sinT])                  # sign-folded
    cq = np.ascontiguousarray(cos * scale).astype(BF)
    sq2 = np.ascontiguousarray(sin2 * scale).astype(BF)
    ck = np.ascontiguousarray(cos).astype(BF)
    sk2 = np.ascontiguousarray(sin2).astype(BF)

    pswap = np.zeros((128, 128), np.float32)
    pswap[(np.arange(128) + 64) % 128, np.arange(128)] = 1.0
    pswap = pswap.astype(BF)

    if mode == 'causal':
        tri = np.where(np.arange(128)[:, None] > np.arange(128)[None, :],
                       np.float32(NEG), np.float32(0.0)).astype(BF)
    elif mode == 'general':
        maskT = np.ascontiguousarray(
            np.asarray(inputs["mask"], np.float32)[0, 0].T).astype(BF)

    def pack_w(wT):
        # [D, CW] -> [128, NDT*CW] wall (d-tile rows side by side)
        return np.ascontiguousarray(
            np.concatenate([wT[dt * 128:(dt + 1) * 128, :]
                            for dt in range(NDT)], axis=1)).astype(BF)

    def pack_h(hsT):
        # [D, S] -> [128, NQC*NDT*QCH] wall, chunk-major then d-tile
        segs = [hsT[dt * 128:(dt + 1) * 128, c * QCH:(c + 1) * QCH]
                for c in range(NQC) for dt in range(NDT)]
        return np.ascontiguousarray(np.concatenate(segs, axis=1)).astype(BF)

    hTb = [pack_h(hs[b].T) for b in range(B)]

    in_maps = []
    for c in range(NCORES):
        b, hg = divmod(c, GPC)
        sl = slice(CW * hg, CW * (hg + 1))
        wq_s = Wq[sl][perm]
        wk_s = Wk[sl][perm]
        m = {
            "hiddenT": hTb[b],
            "wqT": pack_w(wq_s.T),
            "wkT": pack_w(wk_s.T),
            "wvT": pack_w(Wv[sl].T),
            "woT": pack_w(Wo[sl].T),
            "cq": cq, "sq2": sq2, "ck": ck, "sk2": sk2, "pswap": pswap,
        }
        if use_bias:
            m["bqp"] = np.ascontiguousarray(
                bq[sl][perm].reshape(HPC, 128).T).astype(np.float32)
            m["bkp"] = np.ascontiguousarray(
                bk[sl][perm].reshape(HPC, 128).T).astype(np.float32)
            m["bv2"] = bv[sl].reshape(1, CW).astype(np.float32)
            m["bo2"] = bo[sl].reshape(1, CW).astype(np.float32)
        if mode == 'causal':
            m["dmask"] = tri
        elif mode == 'general':
            m["maskT"] = maskT
        in_maps.append(m)
    return in_maps


def _mask_mode(mask):
    mask = np.asarray(mask, np.float32)
    if mask.shape != (1, 1, S, S):
        return 'general'
    m = mask[0, 0]
    if not np.any(m):
        return 'zero'
    expect = np.triu(np.full((S, S), np.float32(NEG)), k=1)
    if np.array_equal(m, expect):
        return 'causal'
    return 'general'


def run_on_cores(inputs, trace=False):
    """Compile+run; returns BassKernelResults."""
    from concourse.bass_utils import run_bass_kernel_spmd
    mode = _mask_mode(inputs["mask"])
    use_bias = any(
        np.any(np.asarray(inputs[k])) for k in ("bq", "bk", "bv", "bo"))
    nc = _get_built(mode, use_bias)
    in_maps = _prep_inputs(inputs, mode, use_bias)
    r = run_bass_kernel_spmd(nc, in_maps, list(range(NCORES)), trace=trace)
    return r


def kernel(**inputs) -> np.ndarray:
    r = run_on_cores(inputs)
    out = np.empty((B, S, D), np.float32)
    for c in range(NCORES):
        b, hg = divmod(c, GPC)
        out[b, :, CW * hg:CW * (hg + 1)] = \
            np.asarray(r.results[c]["out"]).astype(np.float32)
    return out
